# revision 9
# baseline (speedup 1.0000x reference)
"""Distributed causal multi-head attention (QKV projection + flash attention)
for Trainium2, sharded head-parallel across 8 NeuronCores.

Problem: x[2,2048,1024] @ W[1024,3072] + b -> qkv; causal softmax attention
(16 heads, head_dim 64); output [2,2048,16,64].

Sharding: core c handles batch c//4 and the 4 heads 4*(c%4)..4*(c%4)+3.
Each core's output slice is disjoint -> no collectives.

Device kernel (per core, bf16 matmuls with fp32 PSUM accumulation):
  - projection: qT/kT produced transposed ([head-pair 128, S]) with W
    stationary; v natural ([S,64] tiles) with xT stationary.
  - attention per head-pair: scoresT[sk,sq] = kT.T @ qT row-packed 2 heads per
    PE pass into one 2-bank PSUM tile laid out [A w | B w]; DVE drains PSUM
    with a fused (x*scale [+ causal tri mask]) f32->bf16 convert into wide
    SBUF staging tiles; ACT runs few wide bf16 exp instructions (the HW exp
    is ~2x faster on bf16 input and has ~400ns/instr fixed cost, so this
    takes ACT off the critical path); PV matmuls lag LAG groups behind and
    accumulate outT[65, sq] += v'[sk,65].T @ exp[sk,sq] where v' has a ones
    column -> row 64 = softmax denominator.
  - output: unnormalized [4, 65, 2048] f32; host divides by row 64, adds the
    v bias, transposes into the full output.
"""

import numpy as np

NUM_HEAD = 16
HEAD_DIM = 64
HIDDEN = 1024
B, S = 2, 2048
N_CORES = 8
HPC = 4          # heads per core
NCH = 4          # sq chunks of 512
CHW = 512        # chunk width
NT = 16          # sk tiles of 128
KB = 8           # k-dim blocks of 128
NEG = -1.0e9
SCALE = HEAD_DIM ** -0.5
GB = 2           # k-blocks per exp group
LAG = 2          # PV groups of lag behind exp

_CACHE = {}


def _build(repeat=1, gb=GB, lag=LAG):
    import concourse.bacc as bacc
    import concourse.mybir as mybir
    import concourse.tile as tile

    f32 = mybir.dt.float32
    bf16 = mybir.dt.bfloat16
    AF = mybir.ActivationFunctionType
    ALU = mybir.AluOpType

    nc = bacc.Bacc("TRN2", target_bir_lowering=False, debug=False)

    XT = nc.dram_tensor("XT", [HIDDEN, S], bf16, kind="ExternalInput")
    WQK = nc.dram_tensor("WQK", [HIDDEN, 512], bf16, kind="ExternalInput")
    WV = nc.dram_tensor("WV", [HIDDEN, 256], bf16, kind="ExternalInput")
    BQKT = nc.dram_tensor("BQKT", [128, 4], f32, kind="ExternalInput")
    TRIW = nc.dram_tensor("TRIW", [128, 512], f32, kind="ExternalInput")
    OUT = nc.dram_tensor("OUT", [HPC, 65, S], f32, kind="ExternalOutput")

    GW = gb * 1024  # max staged cols per group (2 heads x 512 per block)

    with tile.TileContext(nc) as tc:
        with tc.tile_pool(name="const", bufs=1) as const_pool, \
             tc.tile_pool(name="qkv", bufs=2) as qkv_pool, \
             tc.tile_pool(name="xt", bufs=2) as xt_pool, \
             tc.tile_pool(name="sin", bufs=3) as sin_pool, \
             tc.tile_pool(name="sout", bufs=lag + 2) as sout_pool, \
             tc.tile_pool(name="outs", bufs=4) as out_pool, \
             tc.tile_pool(name="ps_sc", bufs=2, space="PSUM") as ps_sc, \
             tc.tile_pool(name="ps_pr", bufs=2, space="PSUM") as ps_pr, \
             tc.tile_pool(name="ps_pv", bufs=2, space="PSUM") as ps_pv:

            for _rep in range(repeat):
                wqk_sb = const_pool.tile([128, KB, 512], bf16, tag="wqk")
                wv_sb = const_pool.tile([128, KB, 256], bf16, tag="wv")
                bqk_sb = const_pool.tile([128, 4], f32, tag="bqk")
                triw_sb = const_pool.tile([128, 512], f32, tag="triw")

                for kb in range(KB):
                    nc.sync.dma_start(wqk_sb[:, kb, :], WQK[kb * 128:(kb + 1) * 128, :])
                    nc.sync.dma_start(wv_sb[:, kb, :], WV[kb * 128:(kb + 1) * 128, :])
                nc.sync.dma_start(bqk_sb[:], BQKT[:])
                nc.sync.dma_start(triw_sb[:], TRIW[:])

                # qT2/kT2: [pair, 128 (2 heads x 64 d), S]; v: [sk-tile, head, 65]
                qT2 = qkv_pool.tile([128, 2, S], bf16, tag="qT2")
                kT2 = qkv_pool.tile([128, 2, S], bf16, tag="kT2")
                v_sb = qkv_pool.tile([128, NT, HPC, 65], bf16, tag="v")
                nc.vector.memset(v_sb[:, :, :, 64], 1.0)

                # whole-X preload: [128, kb, S]; 8 contiguous 512KB DMAs
                xt_all = xt_pool.tile([128, KB, S], bf16, tag="xt")
                for kb in range(KB):
                    nc.scalar.dma_start(
                        xt_all[:, kb, :], XT[kb * 128:(kb + 1) * 128, :])

                def emit_qkT_group(C, blk):
                    # col-blocks: 0,1 = q pair0/pair1; 2,3 = k pair0/pair1
                    ps = ps_pr.tile([128, CHW], f32, tag="pr")
                    for kb in range(KB):
                        nc.tensor.matmul(
                            ps[:],
                            wqk_sb[:, kb, blk * 128:(blk + 1) * 128],
                            xt_all[:, kb, C * CHW:(C + 1) * CHW],
                            start=(kb == 0), stop=(kb == KB - 1))
                    dest = (qT2 if blk < 2 else kT2)[:, blk % 2,
                                                     C * CHW:(C + 1) * CHW]
                    nc.vector.tensor_scalar_add(dest, ps[:],
                                                bqk_sb[:, blk:blk + 1])

                def emit_v_group(C, rt):
                    t = C * 4 + rt
                    s0 = C * CHW + rt * 128
                    psv = ps_pr.tile([128, 256], f32, tag="pr")
                    for kb in range(KB):
                        nc.tensor.matmul(
                            psv[:],
                            xt_all[:, kb, s0:s0 + 128],
                            wv_sb[:, kb, :],
                            start=(kb == 0), stop=(kb == KB - 1))
                    nc.vector.tensor_copy(v_sb[:, t, :, 0:64], psv[:])

                def proj_pair(C, p):
                    emit_qkT_group(C, p)
                    emit_qkT_group(C, 2 + p)
                    if p == 0:
                        for rt in range(4):
                            emit_v_group(C, rt)

                def attn_pair(C, p):
                    hA, hB = 2 * p, 2 * p + 1
                    nblk = 4 * C + 4
                    pvA = ps_pv.tile([128, CHW], f32, tag="pv")
                    pvB = ps_pv.tile([128, CHW], f32, tag="pv")

                    def blkw(i):
                        m = i - 4 * C
                        return (CHW, False, 0) if m < 0 else (CHW - 128 * m,
                                                              True, 128 * m)

                    def emit_pv(st_out, meta):
                        for (i, o, w, off) in meta:
                            nc.tensor.matmul(
                                pvA[0:65, off:CHW], v_sb[:, i, hA, :],
                                st_out[:, o:o + w],
                                start=(i == 0), stop=(i == nblk - 1))
                            nc.tensor.matmul(
                                pvB[0:65, off:CHW], v_sb[:, i, hB, :],
                                st_out[:, o + w:o + 2 * w],
                                start=(i == 0), stop=(i == nblk - 1))

                    pend = []
                    for g0 in range(0, nblk, gb):
                        blist = range(g0, min(g0 + gb, nblk))
                        st_in = sin_pool.tile([128, GW], bf16, tag="si")
                        st_out = sout_pool.tile([128, GW], bf16, tag="so")
                        o = 0
                        meta = []
                        for i in blist:
                            w, masked, off = blkw(i)
                            sqs = C * CHW + off
                            psM = ps_sc.tile([128, 1024], f32, tag="sc")
                            nc.tensor.matmul(
                                psM[:, 0:w],
                                kT2[0:64, p, i * 128:(i + 1) * 128],
                                qT2[0:64, p, sqs:sqs + w],
                                start=True, stop=True, tile_position=(0, 0))
                            nc.tensor.matmul(
                                psM[:, CHW:CHW + w],
                                kT2[64:128, p, i * 128:(i + 1) * 128],
                                qT2[64:128, p, sqs:sqs + w],
                                start=True, stop=True, tile_position=(64, 0))
                            if masked:
                                nc.vector.scalar_tensor_tensor(
                                    st_in[:, o:o + w], psM[:, 0:w], SCALE,
                                    triw_sb[:, 0:w], ALU.mult, ALU.add)
                                nc.vector.scalar_tensor_tensor(
                                    st_in[:, o + w:o + 2 * w],
                                    psM[:, CHW:CHW + w],
                                    SCALE, triw_sb[:, 0:w], ALU.mult, ALU.add)
                            else:
                                nc.vector.tensor_scalar_mul(
                                    st_in[:, o:o + 2 * w], psM[:, 0:2 * w],
                                    SCALE)
                            meta.append((i, o, w, off))
                            o += 2 * w
                        nc.scalar.activation(st_out[:, 0:o], st_in[:, 0:o],
                                             AF.Exp)
                        pend.append((st_out, meta))
                        if len(pend) > lag:
                            emit_pv(*pend.pop(0))
                    for ent in pend:
                        emit_pv(*ent)

                    oA = out_pool.tile([128, CHW], f32, tag="o")
                    oB = out_pool.tile([128, CHW], f32, tag="o")
                    nc.vector.tensor_copy(oA[0:65, :], pvA[0:65, :])
                    nc.vector.tensor_copy(oB[0:65, :], pvB[0:65, :])
                    nc.sync.dma_start(OUT[hA, :, C * CHW:(C + 1) * CHW],
                                      oA[0:65, :])
                    nc.sync.dma_start(OUT[hB, :, C * CHW:(C + 1) * CHW],
                                      oB[0:65, :])

                for C in range(NCH):
                    for p in range(2):
                        proj_pair(C, p)
                        attn_pair(C, p)

    nc.compile()
    return nc


def _get_nc(repeat=1):
    key = ("nc", repeat, GB, LAG)
    if key not in _CACHE:
        _CACHE[key] = _build(repeat)
    return _CACHE[key]


def _prep_inputs(x, W, b):
    import ml_dtypes
    bf16 = ml_dtypes.bfloat16

    x = np.asarray(x, dtype=np.float32)
    W = np.asarray(W, dtype=np.float32)
    b = np.asarray(b, dtype=np.float32)

    W4 = W.reshape(HIDDEN, 3, NUM_HEAD, HEAD_DIM)
    b4 = b.reshape(3, NUM_HEAD, HEAD_DIM)

    xT = [np.ascontiguousarray(x[bi].T).astype(bf16) for bi in range(B)]

    tri = np.where(np.arange(128)[None, :] >= np.arange(128)[:, None],
                   np.float32(0.0), np.float32(NEG)).astype(np.float32)
    triw = np.concatenate([tri, np.zeros((128, 384), np.float32)], axis=1)

    in_maps = []
    for c in range(N_CORES):
        bi, g = divmod(c, HPC)
        heads = [4 * g + j for j in range(HPC)]
        wqk = np.concatenate(
            [W4[:, 0, h, :] for h in heads] + [W4[:, 1, h, :] for h in heads],
            axis=1)  # [1024, 512]
        wv = np.concatenate([W4[:, 2, h, :] for h in heads], axis=1)  # [1024,256]
        bqkt = np.stack(
            [np.concatenate([b4[0, heads[0]], b4[0, heads[1]]]),
             np.concatenate([b4[0, heads[2]], b4[0, heads[3]]]),
             np.concatenate([b4[1, heads[0]], b4[1, heads[1]]]),
             np.concatenate([b4[1, heads[2]], b4[1, heads[3]]])],
            axis=1)  # [128, 4]
        in_maps.append({
            "XT": xT[bi],
            "WQK": np.ascontiguousarray(wqk).astype(bf16),
            "WV": np.ascontiguousarray(wv).astype(bf16),
            "BQKT": np.ascontiguousarray(bqkt),
            "TRIW": triw,
        })
    return in_maps, b4


def kernel(x, W, b):
    from concourse.bass_utils import run_bass_kernel_spmd

    in_maps, b4 = _prep_inputs(x, W, b)
    nc = _get_nc()
    res = run_bass_kernel_spmd(nc, in_maps, core_ids=list(range(N_CORES)))

    out = np.empty((B, S, NUM_HEAD, HEAD_DIM), dtype=np.float32)
    for c in range(N_CORES):
        bi, g = divmod(c, HPC)
        u = res.results[c]["OUT"]               # [4, 65, 2048]
        o = u[:, :64, :] / u[:, 64:65, :]        # [4, 64, 2048]
        out[bi, :, 4 * g:4 * g + 4, :] = o.transpose(2, 0, 1)
    out += b4[2].reshape(1, 1, NUM_HEAD, HEAD_DIM)
    return out
